# revision 24
# baseline (speedup 1.0000x reference)
"""Trainium2 Bass kernel for the AnalyticalBoundedLineAttractor problem.

Reference semantics (per step, per sample):
    z = x @ W.T + b;  m = (z > 0);  A = diag(m) @ W - I;  c = m * b
    x_next = expm(A*dt) @ x + (expm(A*dt) - I) @ pinv(A) @ c

Scheme: K=2 Taylor of the augmented matrix exponential (lam = exp(-dt)):
    p0  = dt*(W x + b);  v1 = lam*relu(p0)
    v2  = (v1 > 0) * ((dt/2) W v1 + lam*(dt^2/2) b)
    x'  = lam*x + v1 + v2

This is a LATENCY-bound problem: all 8 cores run the same serial
99-step recurrence, so wall time == per-step critical-path length.
Two one-step lags cut the chain from 4 engine-ops (~912 ns baseline)
to 2 (~590 ns):
  * the correction matmul B uses the PREVIOUS step's v1, and
  * v2 enters the state one step late, through the auxiliary state
    Xg_{t+1} = lam*x_{t+1} + v2_{t-1}  (x_{t+1} = Xg_t + v1_t).
Both lags are O(dt^3)-per-step perturbations, the same order as the
K=2 truncation (numpy check vs the expm/pinv reference: 1.8e-3 rel
err with fp16; the gate is 2e-2).

The states are stored PRE-SCALED so every combine is either a plain
tensor-tensor add (Pool engine) or a single DVE STT, and the per-step
lam scalings ride in the (static) weight blocks:
    G == lam*Xg   (history),   Vh == lam^2*v1   (relu output, history)
    A  = lam^2*dt*(W x + b)   -> relu(A) = Vh directly (scale=1)
    B' = lam*((dt/2) W v1+c2) -> v2L = (Vh>0)*B' = lam*v2

Per-step engine schedule (chain = ACT relu -> PE matmul -> ACT relu):
    ACT   : Vh_t = relu(A_t)                                   [CHAIN]
    PE    : A_{t+1} = w0@G_t (start) + w1@Vh_t (stop)          [CHAIN]
            B'_{t+1} = w2@Vh_t
    DVE   : r_t = lam*G_t + v2L_{t-1}  (frame 0; operands old)
            v2L_t = (Vh_t > 0) * B'_t
    Pool  : G_{t+1} = Vh_t + r_t    (one hop after the relu)
Weight blocks (three 64-col blocks in one SBUF tile; each matmul
LDWEIGHTs its own slice):  w0 = lam*dt*W.T (64 rows),
w1 = dt*W.T | row64 = lam^2*dt*b,  w2 = (dt/2lam)*W.T | row64 = lam*c2.
G/Vh are append-only histories, DMA-streamed out during the loop; the
host reconstructs x_{t+1} = G_t/lam + Vh_t/lam^2.
Per-core 32 samples, D=64 on partitions, fp16 state, fp32 PSUM.
"""

import math
import sys

import numpy as np

try:
    from concourse.bass_utils import run_bass_kernel_spmd
except ImportError:
    sys.path.insert(0, "/opt/trn_rl_repo")
    from concourse.bass_utils import run_bass_kernel_spmd

import concourse.bacc as bacc
import concourse.mybir as mybir
import concourse.tile as tile

DT = 0.05
T_STEPS = 100
DIM = 64
BATCH = 256
N_CORES = 8
BL = BATCH // N_CORES  # 32 samples per core
NT = T_STEPS - 1  # 99 loop steps
LAM = math.exp(-DT)
F32 = mybir.dt.float32
F16 = mybir.dt.float16

_CACHE = {}


def _build_nc():
    nc = bacc.Bacc(None, target_bir_lowering=False)
    x0_ext = nc.declare_dram_parameter("x0h", [DIM, BL], F16, isOutput=False)
    wts_ext = nc.declare_dram_parameter("wth", [DIM + 1, 3 * DIM], F16, isOutput=False)
    ones_ext = nc.declare_dram_parameter("oneh", [1, NT * BL], F16, isOutput=False)
    g_ext = nc.declare_dram_parameter("gh", [DIM, NT * BL], F16, isOutput=True)
    v_ext = nc.declare_dram_parameter("vh", [DIM, NT * BL], F16, isOutput=True)

    OP = mybir.AluOpType
    ACTF = mybir.ActivationFunctionType

    with tile.TileContext(nc) as tc:
        with (
            tc.tile_pool(name="sb", bufs=1) as sb,
            tc.tile_pool(name="ps", bufs=2, space="PSUM") as ps,
        ):
            wts = sb.tile([DIM + 1, 3 * DIM], F16)
            x0S = sb.tile([DIM + 1, BL], F16)  # lam^2*x0 | row64 = 1
            # append-only histories (fresh slot per step -> single-producer
            # tiles, one wait condition per consumer, no WAR sems in front
            # of the chain-critical relu).  Vh's bias row (ones) comes in
            # by DMA -- a full-row memset costs ~2.7us and gates the start.
            Vh = sb.tile([DIM + 1, NT * BL], F16)  # row DIM = 1 (bias row)
            Gh = sb.tile([DIM, (NT + 1) * BL], F16)
            v2L = [sb.tile([DIM, BL], F16, name=f"v2L_{k}") for k in range(2)]
            rv = sb.tile([DIM, BL], F16)

            # the host sends x0 pre-scaled by lam^2, so it serves directly
            # as both the A_0 matmul rhs and G_0 -- no on-device copies
            nc.sync.dma_start(wts[:], wts_ext[:])
            nc.scalar.dma_start(x0S[0:DIM, :], x0_ext[:])
            nc.scalar.dma_start(Gh[:, 0:BL], x0_ext[:])
            nc.sync.dma_start(Vh[DIM : DIM + 1, :], ones_ext[:])
            nc.vector.memset(x0S[DIM : DIM + 1, :], 1.0)
            nc.vector.memset(v2L[0][:], 0.0)
            nc.vector.memset(v2L[1][:], 0.0)

            w0 = wts[0:DIM, 0:DIM]  # lam^2*dt*W.T      (G part, no bias)
            w1 = wts[:, DIM : 2 * DIM]  # lam*dt*W.T  | row64 = lam^3*dt*b
            w2 = wts[:, 2 * DIM : 3 * DIM]  # dt/(2lam)*W.T | row64 = lam*c2

            A_cur = ps.tile([DIM, BL], F32, name="A")
            nc.tensor.matmul(A_cur[:], w1, x0S[:], start=True, stop=True)

            for t in range(NT):
                sV = Vh[:, t * BL : (t + 1) * BL]
                sG = Gh[:, t * BL : (t + 1) * BL]

                # [CHAIN] Vh_t = relu(A_t)
                nc.scalar.activation(sV[0:DIM, :], A_cur[:], ACTF.Relu)

                # A_{t+1} = w0@G_t + w1@Vh_t + bias.  The G part issues as
                # soon as G_t lands (mid-relu) and drains; the Vh part
                # issues at the relu sem and is the only chain matmul.
                A_nxt = ps.tile([DIM, BL], F32, name="A")
                nc.tensor.matmul(A_nxt[:], w0, sG, start=True, stop=False)
                nc.tensor.matmul(A_nxt[:], w1, sV, start=False, stop=True)

                # B'_t = lam*((dt/2)W v1_{t-1} + c2): reads the PREVIOUS
                # step's Vh, so it's ready at step start and never blocks
                # the chain A-matmuls on the in-order PE.
                if t > 0:
                    sVp = Vh[:, (t - 1) * BL : t * BL]
                    B_cur = ps.tile([DIM, BL], F32, name="B")
                    nc.tensor.matmul(B_cur[:], w2, sVp, start=True, stop=True)

                # r_t = lam*G_t + v2L_{t-1}: both operands are from earlier
                # steps -- runs at step start, fully off the chain.
                if t < NT - 1:
                    nc.vector.scalar_tensor_tensor(
                        rv[:], sG, LAM, v2L[(t - 1) % 2][:],
                        op0=OP.mult, op1=OP.add,
                    )
                    # G_{t+1} = Vh_t + r_t  (= lam*Xg_{t+1})
                    sG1 = Gh[:, (t + 1) * BL : (t + 2) * BL]
                    nc.vector.tensor_tensor(sG1, sV[0:DIM, :], rv[:], op=OP.add)

                # v2L_t = (Vh_t > 0) * B'_t  (read by r_{t+1})
                if t > 0:
                    nc.vector.scalar_tensor_tensor(
                        v2L[t % 2][:], sV[0:DIM, :], 0.0, B_cur[:],
                        op0=OP.is_gt, op1=OP.mult,
                    )

                A_cur = A_nxt

                # stream finished history chunks; small tail chunk
                bounds = {17: 0, 37: 18, 57: 38, 77: 58, 94: 78, 98: 95}
                if t in bounds:
                    lo, hi = bounds[t] * BL, (t + 1) * BL
                    nc.sync.dma_start(g_ext[:, lo:hi], Gh[:, lo:hi])
                    nc.scalar.dma_start(v_ext[:, lo:hi], Vh[0:DIM, lo:hi])

    nc.compile()
    return nc


def _host_weights(W, b):
    """Three stationary blocks (DIM+1, 3*DIM) fp16; fp64 math then cast."""
    W64 = W.astype(np.float64)
    b64 = b.astype(np.float64)
    c2 = LAM * (DT**2 / 2) * b64
    # A is carried as lam^3*dt*(W x + b) so that relu(A) = lam^2*v1 = Vh
    # (v1 = lam*relu(dt*z)); the per-state scales fold into the blocks:
    # rhs G = lam*Xg, rhs Vh = lam^2*v1.
    wts = np.zeros((DIM + 1, 3 * DIM), np.float64)
    wts[0:DIM, 0:DIM] = LAM**2 * DT * W64.T
    wts[0:DIM, DIM : 2 * DIM] = LAM * DT * W64.T
    wts[DIM, DIM : 2 * DIM] = LAM**3 * DT * b64
    wts[0:DIM, 2 * DIM : 3 * DIM] = (DT / (2 * LAM)) * W64.T
    wts[DIM, 2 * DIM : 3 * DIM] = LAM * c2
    return np.ascontiguousarray(wts.astype(np.float16))


def _run_device(x0, W, b, **spmd_kwargs):
    if "nc" not in _CACHE:
        _CACHE["nc"] = _build_nc()
    nc = _CACHE["nc"]

    wts = _host_weights(W, b)
    ones = np.ones((1, NT * BL), np.float16)
    in_maps = []
    for i in range(N_CORES):
        shard = np.ascontiguousarray(
            (LAM**2 * x0[i * BL : (i + 1) * BL].astype(np.float64))
            .T.astype(np.float16)
        )
        in_maps.append({"x0h": shard, "wth": wts, "oneh": ones})

    return run_bass_kernel_spmd(
        nc, in_maps, core_ids=list(range(N_CORES)), **spmd_kwargs
    )


def kernel(initial_position, W, b):
    x0 = np.asarray(initial_position, np.float32)
    W = np.asarray(W, np.float32)
    b = np.asarray(b, np.float32)

    res = _run_device(x0, W, b)

    out = np.empty((BATCH, T_STEPS, DIM), np.float32)
    inv_lam = 1.0 / LAM
    inv_lam2 = 1.0 / LAM**2
    for i in range(N_CORES):
        gh = res.results[i]["gh"].astype(np.float32)  # (DIM, NT*BL)
        vh = res.results[i]["vh"].astype(np.float32)
        # x_{t+1} = Xg_t + v1_t = G_t/lam + Vh_t/lam^2
        xt = (inv_lam * gh + inv_lam2 * vh).reshape(DIM, NT, BL).transpose(2, 1, 0)
        out[i * BL : (i + 1) * BL, 0] = x0[i * BL : (i + 1) * BL]
        out[i * BL : (i + 1) * BL, 1:] = xt
    return out
